# revision 27
# baseline (speedup 1.0000x reference)
"""LoRA layer kernel for Trainium2, 8-core data-parallel.

out = x @ W.T + 2.0 * ((x @ B) @ A)
  x: (4, 4096, 4096) f32, W: (4096, 4096), A: (16, 4096), B: (4096, 16)

Strategy: fold the LoRA term into the weight on the host
(WF = W.T + 2*B@A, exact same math, 0.5 GFLOP of numpy), so the device
kernel is a single dense GEMM out = x @ WF. Flatten x to (16384, 4096)
rows, shard rows across 8 cores (2048 rows each), replicate WF. All
matmul operands bf16 (PSUM accumulation fp32; rel err ~2e-3 vs the
2e-2 gate).

Per core, single x-resident block (2048 rows = 128 KB/partition bf16):
  - warmup: 8 junk matmuls on a memset tile (no DMA dependency) so the
    PE HAM clock ramps to 8/8 while the first x/W blocks are in flight.
  - x loads m-split into 512-col quarters, laid out [k4][mq][kk][m] so
    each (k-quad, m-quarter) block is one flat 512KB DMA.
  - DMA ring capabilities (measured): gpsimd SWDGE ~210 GB/s sustained
    but first-DMA ~11.4us; HWDGE rings (sync, scalar) start ~9-10us
    but only ~58-70 GB/s each. Startup need-order schedule: W00/W02/
    W05 on sync, x00 (kk-granular) + W04/W06/W07 on scalar, x01/x02
    halves + W01/W03 + remaining x on gpsimd. W quads are atomic deps
    (readers of a multi-write tile wait its LAST write) so they are
    never split across DMAs; x chunks unlock progressively. W(oc>=1)
    alternates sync/scalar (wpool frees pace them k4-aligned); out
    rides gpsimd, which is idle once x drains (~85us).
  - main GEMM: per o-chunk (512 cols, one PSUM bank), 4 m-quarters of
    4 PSUM banks each (4+4 double buffering: quarter q+1's banks were
    drained during quarter q-1, so bank reuse never stalls the PE);
    x-tile stationary, WF streamed as k-quads [128, 4*512].
  - last quarter (oc7,q3) runs mi-outer/k-inner (all data resident) so
    each PSUM bank finishes 32 MMs before the next starts: drain+store
    pipelines per-mi and the tail is two 128x256 copy+DMA halves
    (~1.5us) instead of 4 copies + a 1MB DMA (~8us).
"""

import sys

if "/opt/trn_rl_repo" not in sys.path:
    sys.path.insert(0, "/opt/trn_rl_repo")

import numpy as np
import ml_dtypes

import concourse.bass as bass
import concourse.mybir as mybir
import concourse.tile as tile

N_CORES = 8
D = 4096
RANK = 16
ROWS_TOTAL = 4 * 4096          # 16384
M = ROWS_TOTAL // N_CORES      # 2048 rows per core
P = 128
KT = D // P                    # 32 k-tiles
OC = 512                       # o-chunk width (one PSUM bank)
N_OC = D // OC                 # 8
MT = M // P                    # 16 m-tiles
MQ = 4                         # m-tiles per quarter (PSUM banks)
NQ = MT // MQ                  # 4 quarters
KQ = 4                         # k-tiles per W quad DMA
N_KQ = KT // KQ                # 8

F32 = mybir.dt.float32
BF16 = mybir.dt.bfloat16
BF16_NP = ml_dtypes.bfloat16

N_WARMUP = 14   # junk MMs bridge memset (~8.4us) to first data (~14.5us)


def split_wide_waits(nc, max_waits=1):
    """walrus in this container rejects >1 sync wait per instruction;
    move excess waits onto preceding same-engine NoOps."""
    n_split = 0
    for f in nc.m.functions:
        for bb in f.blocks:
            new_insts = []
            for inst in bb.instructions:
                si = getattr(inst, "sync_info", None)
                if si is not None and si.on_wait and len(si.on_wait) > max_waits:
                    waits = list(si.on_wait)
                    keep = waits[-max_waits:]
                    extra = waits[:-max_waits]
                    for i in range(0, len(extra), max_waits):
                        chunk = extra[i:i + max_waits]
                        nop = mybir.InstNoOp(
                            name=f"{inst.name}_wsplit{i}",
                            sync_info=mybir.SyncInfo(on_wait=chunk, on_update=[]),
                            bass_nofuse=True,
                            engine=inst.engine,
                        )
                        new_insts.append(nop)
                        n_split += 1
                    si.on_wait = keep
                new_insts.append(inst)
            bb.instructions[:] = new_insts
    return n_split


def build_program():
    nc = bass.Bass()
    xt = nc.declare_dram_parameter("xt", [D, M], BF16, isOutput=False)
    wf = nc.declare_dram_parameter("wf", [D, D], BF16, isOutput=False)
    out = nc.declare_dram_parameter("out", [M, D], F32, isOutput=True)

    with tile.TileContext(nc) as tc:
        with (
            tc.tile_pool(name="xpool", bufs=1) as xpool,
            tc.tile_pool(name="wpool", bufs=9) as wpool,
            tc.tile_pool(name="opool", bufs=4) as opool,
            tc.tile_pool(name="cpool", bufs=1) as cpool,
            tc.tile_pool(name="ppool", bufs=8, space="PSUM") as ppool,
        ):
            # HAM warmup on a memset tile: no DMA dependency, so the PE
            # is busy (and ramping to 8/8) from ~t0 while x/W stream in.
            jt = cpool.tile([P, OC], BF16, tag="junk_src")
            nc.vector.memset(jt[:], 0.0)
            junk = ppool.tile([P, OC], F32, tag="acc", name="junk")
            for i in range(N_WARMUP):
                nc.tensor.matmul(
                    junk[:],
                    jt[:, :P],
                    jt[:],
                    start=(i == 0),
                    stop=(i == N_WARMUP - 1),
                )

            # x fully resident, laid out [k4][mq][kk][m] so each
            # k-quad x m-quarter block is flat-contiguous: one 512KB DMA
            # per block (32 total), arriving at exactly the granularity
            # the PE consumes (16 main MMs per block).
            xall = xpool.tile([P, KT * M], BF16, tag="x")

            def xsl(k, c0, cw):
                k4, kk = divmod(k, KQ)
                mq, d = divmod(c0, OC)
                assert d + cw <= OC
                base = ((k4 * NQ + mq) * KQ + kk) * OC + d
                return xall[:, base: base + cw]

            def x_load(mq, k4, eng, kk0=0, nkk=KQ):
                base = ((k4 * NQ + mq) * KQ + kk0) * OC
                eng.dma_start(
                    xall[:, base: base + nkk * OC].rearrange(
                        "p (b c) -> p b c", b=nkk),
                    xt[(k4 * KQ + kk0) * P:(k4 * KQ + kk0 + nkk) * P,
                       mq * OC:(mq + 1) * OC].rearrange(
                           "(b p) c -> p b c", p=P),
                )

            def w_dma(wtile, oc, k4, eng, kk0=0, nkk=KQ):
                eng.dma_start(
                    wtile[:, kk0 * OC:(kk0 + nkk) * OC].rearrange(
                        "p (b c) -> p b c", b=nkk),
                    wf[(k4 * KQ + kk0) * P:(k4 * KQ + kk0 + nkk) * P,
                       oc * OC:(oc + 1) * OC].rearrange(
                           "(b p) c -> p b c", p=P),
                )

            def w_quad(oc, k4, eng=None):
                wtile = wpool.tile([P, KQ * OC], BF16, tag="wt")
                w_dma(wtile, oc, k4, eng or nc.sync)
                return wtile

            # Ring capabilities (measured): gpsimd SWDGE ~210 GB/s;
            # each HWDGE ring (sync, scalar) only ~58 GB/s. Quarter 0
            # needs x at 148 GB/s + W(oc0) at 148 GB/s, so: all x on
            # gpsimd in need order, W(oc0) split 3 ways with quads
            # 0/3/6 interleaved into the gpsimd stream at their need
            # points; later W alternates sync/scalar; out rides gpsimd
            # (idle once x drains).
            # buffer allocation in k4 order (so W(oc1,k4) later waits on
            # the free of wtiles0[k4-1] — perfectly pipelined), DMAs
            # issued in per-ring need order.
            wtiles0 = [wpool.tile([P, KQ * OC], BF16, tag="wt",
                                  name=f"wt0_{k4}")
                       for k4 in range(N_KQ)]
            # Ring start latencies (measured): HWDGE sync/scalar deliver
            # from ~9-10us (burst ~113 GB/s solo, ~70 sustained); the
            # gpsimd SWDGE ring starts ~11.4us then sustains ~210 GB/s.
            # W quads are atomic deps (readers of a multi-write tile
            # wait for its LAST write), so never split them; x00 kk
            # chunks unlock progressively (xall region deps work).
            # sync: W00 first (gates the first real MM, ~14.5us), then
            # W02, W07. scalar: x00 chunks, then W04, W06. gpsimd: the
            # rest of x in need order with W01/W03/W05 interleaved.
            w_dma(wtiles0[0], 0, 0, nc.sync)
            w_dma(wtiles0[2], 0, 2, nc.sync)
            w_dma(wtiles0[5], 0, 5, nc.sync)
            for kk in range(KQ):
                x_load(0, 0, nc.scalar, kk0=kk, nkk=1)
            w_dma(wtiles0[4], 0, 4, nc.scalar)
            w_dma(wtiles0[6], 0, 6, nc.scalar)
            w_dma(wtiles0[7], 0, 7, nc.scalar)
            # gpsimd starts ~11.4us: x01/x02 split in halves so their
            # first halves land before the PE reaches k4=1/2
            for half in (0, 1):
                x_load(0, 1, nc.gpsimd, kk0=2 * half, nkk=2)
            w_dma(wtiles0[1], 0, 1, nc.gpsimd)
            for half in (0, 1):
                x_load(0, 2, nc.gpsimd, kk0=2 * half, nkk=2)
            w_dma(wtiles0[3], 0, 3, nc.gpsimd)
            for k4 in range(3, N_KQ):
                x_load(0, k4, nc.gpsimd)
            for mq in range(1, NQ):
                for k4 in range(N_KQ):
                    x_load(mq, k4, nc.gpsimd)

            def quarter_mms(wtiles, q, psq):
                for k4 in range(N_KQ):
                    for kk in range(KQ):
                        k = KQ * k4 + kk
                        for mi in range(MQ):
                            mt = q * MQ + mi
                            nc.tensor.matmul(
                                psq[mi][:],
                                xsl(k, mt * P, P),
                                wtiles[k4][:, kk * OC:(kk + 1) * OC],
                                start=(k == 0),
                                stop=(k == KT - 1),
                            )

            def out_dma(eng, oc, q, ot, mi=None):
                if mi is None:
                    eng.dma_start(
                        out[q * MQ * P:(q + 1) * MQ * P,
                            oc * OC:(oc + 1) * OC].rearrange(
                                "(b p) c -> p b c", p=P),
                        ot.rearrange("p (b c) -> p b c", b=MQ),
                    )
                else:
                    mt = q * MQ + mi
                    eng.dma_start(
                        out[mt * P:(mt + 1) * P, oc * OC:(oc + 1) * OC],
                        ot[:, mi * OC:(mi + 1) * OC],
                    )

            for oc in range(N_OC):
                wtiles = wtiles0 if oc == 0 else [
                    w_quad(oc, k4, nc.sync if k4 % 2 == 0 else nc.scalar)
                    for k4 in range(N_KQ)
                ]
                for q in range(NQ):
                    psq = [
                        ppool.tile([P, OC], F32, tag="acc",
                                   name=f"ps_{oc}_{q}_{mi}")
                        for mi in range(MQ)
                    ]
                    last = (oc == N_OC - 1 and q == NQ - 1)
                    ot = opool.tile([P, MQ * OC], F32, tag="ot")
                    if not last:
                        quarter_mms(wtiles, q, psq)
                        for mi in range(MQ):
                            nc.vector.tensor_copy(
                                ot[:, mi * OC:(mi + 1) * OC], psq[mi][:]
                            )
                        # out rides the gpsimd ring: x occupies it only
                        # for the first ~85us and HWDGE stays free for W
                        out_dma(nc.gpsimd, oc, q, ot)
                    else:
                        # mi-outer so each PSUM bank completes its 32-MM
                        # accumulation early; drain+store pipeline per-mi
                        # cuts the kernel tail to one copy + one 256KB DMA.
                        for mi in range(MQ):
                            mt = q * MQ + mi
                            for k4 in range(N_KQ):
                                for kk in range(KQ):
                                    k = KQ * k4 + kk
                                    nc.tensor.matmul(
                                        psq[mi][:],
                                        xsl(k, mt * P, P),
                                        wtiles[k4][:, kk * OC:(kk + 1) * OC],
                                        start=(k == 0),
                                        stop=(k == KT - 1),
                                    )
                            if mi < MQ - 1:
                                nc.vector.tensor_copy(
                                    ot[:, mi * OC:(mi + 1) * OC], psq[mi][:]
                                )
                                seng = nc.gpsimd if mi % 2 == 0 else nc.scalar
                                out_dma(seng, oc, q, ot, mi=mi)
                            else:
                                # split the very last drain 4 ways so the
                                # kernel tail is one 128x128 copy + 64KB DMA
                                mt = q * MQ + mi
                                H = OC // 4
                                tengs = [nc.sync, nc.scalar,
                                         nc.sync, nc.scalar]
                                for h in range(4):
                                    nc.vector.tensor_copy(
                                        ot[:, mi * OC + h * H:
                                           mi * OC + (h + 1) * H],
                                        psq[mi][:, h * H:(h + 1) * H],
                                    )
                                    tengs[h].dma_start(
                                        out[mt * P:(mt + 1) * P,
                                            oc * OC + h * H:
                                            oc * OC + (h + 1) * H],
                                        ot[:, mi * OC + h * H:
                                           mi * OC + (h + 1) * H],
                                    )

    split_wide_waits(nc)
    return nc


_NC_CACHE = [None]


def kernel(x, weight, lora_A, lora_B):
    from concourse.bass_utils import run_bass_kernel_spmd

    x = np.asarray(x, dtype=np.float32)
    weight = np.asarray(weight, dtype=np.float32)
    lora_A = np.asarray(lora_A, dtype=np.float32)
    lora_B = np.asarray(lora_B, dtype=np.float32)

    x2 = x.reshape(ROWS_TOTAL, D)
    # fold LoRA into the weight: out = x @ (W.T + 2*B@A), exact rewrite
    wf = np.ascontiguousarray(
        weight.T + 2.0 * (lora_B @ lora_A)
    ).astype(BF16_NP)

    in_maps = []
    for c in range(N_CORES):
        xt_c = np.ascontiguousarray(
            x2[c * M:(c + 1) * M].T
        ).astype(BF16_NP)
        in_maps.append({"xt": xt_c, "wf": wf})

    if _NC_CACHE[0] is None:
        _NC_CACHE[0] = build_program()
    nc = _NC_CACHE[0]

    res = run_bass_kernel_spmd(nc, in_maps, list(range(N_CORES)))
    out = np.concatenate(
        [res.results[c]["out"] for c in range(N_CORES)], axis=0
    )
    return out.reshape(x.shape)
